# revision 15
# baseline (speedup 1.0000x reference)
"""Trainium2 Bass kernel for nn_CompressedSensingConvolutional.

Problem: 200 FISTA iterations of
    re    = conv_transpose(x - conv(y_tmp, w_conv, stride 8, SAME) - b_conv,
                           w_ct, stride 8, SAME) + b_ct
    w     = y_tmp - re
    y_new = soft_threshold(w, lam)        (per-sample lam)
    y_tmp = y_new + beta_n (y_new - y_last)
with x: (64,9,9,3), output y_new: (64,72,72,3).

Structure exploited (all exact, no approximations):
  * conv_transpose with 5x5 kernel / stride 8 writes NON-overlapping 5x5
    patches at output rows/cols 8I+a, a in 0..4. Positions with row%8>4 or
    col%8>4 never receive an update, so (given b_ct=0 there) they stay 0
    bitwise forever. The live state is a 45x45x3 = [75=(a,b,ci), 9x9 blocks]
    phase-space grid of 6075 values per sample.
  * With y==0, w = -c where c = At(x - b_conv) + b_ct. If |c| <= lam
    elementwise, soft_threshold returns exactly 0 and the state never
    leaves 0: the sample's output is exactly zero. Only samples with
    max|c| > lam ("active") need the 200-iteration loop at all.
  * conv(y) restricted to the live grid is a 5x5 conv over the 9x9 block
    grid with 75 input channels -> 3 outputs = 25 shift-matmuls (K=75, M=3,
    N=81) accumulated in PSUM.
  * FISTA momentum commutes with the linear conv: A(y_new + b(y_new-y_last))
    is formed in the tiny 3x81 z-space from per-iteration conv outputs,
    so y_tmp is never materialized for the conv input.

Each active sample runs on its own NeuronCore (8 cores; extra actives are
handled in additional device rounds). The 200 iterations run inside a
tc.For_i HARDWARE LOOP (4 iterations per body; y ping-pong stays static)
with per-iteration momentum coefficients read from an SBUF table via
dynamic slices. The static program is ~180 instructions instead of ~7400
fully unrolled, which removes the NEFF-size-proportional per-launch cost
that dominated the unrolled version, while on-device time stays at the
cost-model floor (~5.6 us/iteration before column-packing credit).
"""

import math
import os
import sys

import numpy as np

for _p in ("/opt/trn_rl_repo", "/root/.axon_site/_ro/trn_rl_repo"):
    if os.path.isdir(_p) and _p not in sys.path:
        sys.path.insert(0, _p)

N_ITERS = 200
N_CORES = 8
HW = 72
LOW = 9
C = 3
F = 75          # (a,b,ci): 5*5*3 live phase-space channels
NP2 = 13        # padded block grid (9 + 2 on each side)
NB = 9          # block grid
NPOS = NB * NB  # 81
UNROLL = 4      # iterations per hw-loop body (even: keeps ping-pong static)


def _betas(n_iters):
    """beta_n = (t_n - 1)/t_{n+1}, bit-exact fp32 mirror of the reference."""
    one, two, four = np.float32(1.0), np.float32(2.0), np.float32(4.0)
    t = np.float32(1.0)
    out = []
    for _ in range(n_iters):
        t_n = (one + np.sqrt(one + four * t * t)) / two
        out.append(float((t - one) / t_n))
        t = t_n
    return out


_DEV_CACHE = {}


def _build_device(n_iters, nbet_min=2, no_loop=False):
    """Per-core FISTA program (SPMD, same code all cores).

    Layout: one active sample per core. State y_tmp lives in a padded
    [75, 13x13] fp32 SBUF tile. Per iteration:
      z-phase : 25 shift-matmuls (5x5 phase conv, K=75, M=3, N=81) spread
                over 4 PSUM column groups (tile_position col packing) so 4
                streams overlap on the PE array; +1 matmul adds bx.
      reduce  : group partials summed into zy's zcomb rows - 2 copies
                (1 on ACT) + 3 adds on DVE.
      At-phase: w = Wcomb^T zy_int as ONE K=100 matmul (identity rows fold
                in y_tmp, Wr rows apply At, ones row folds -b_ct).
      soft    : cl = clamp(w, +-lam); y_new = w - cl   (DVE, reads PSUM)
      momentum: y_tmp' = (1+beta_n) y_new - beta_n y_last, with
                e = beta_n*y_last issued early (overlaps the z-phase) and
                the rest fused into one scalar_tensor_tensor op.

    ``nbet_min`` pads the beta-table input so different n_iters builds can
    share identical input shapes (used by the timing harness).
    ``no_loop`` forces a fully python-unrolled build (used for the
    TimelineSim cost model, which cannot simulate register branches).
    """
    key = (n_iters, nbet_min, no_loop)
    if key in _DEV_CACHE:
        return _DEV_CACHE[key]

    import concourse.bacc as bacc
    import concourse.mybir as mybir
    from concourse.bass import ds
    from concourse.tile import TileContext

    f32 = mybir.dt.float32
    Alu = mybir.AluOpType

    betas = [float(b) for b in _betas(max(n_iters, 1))]
    n_loop = 0 if no_loop else (n_iters // UNROLL) * UNROLL
    nbet = max(2 * n_iters, 2, nbet_min)

    nc = bacc.Bacc(trn_type="TRN2")
    wc_d = nc.dram_tensor("wc", [F, 75], f32, kind="ExternalInput")
    wcomb_d = nc.dram_tensor("wcomb", [100, F], f32, kind="ExternalInput")
    bx_d = nc.dram_tensor("bx", [C, NPOS], f32, kind="ExternalInput")
    i3_d = nc.dram_tensor("i3", [C, C], f32, kind="ExternalInput")
    lam_d = nc.dram_tensor("lam2", [F, 2], f32, kind="ExternalInput")
    bet_d = nc.dram_tensor("bet", [F, nbet], f32, kind="ExternalInput")
    y_d = nc.dram_tensor("y", [F, NPOS], f32, kind="ExternalOutput")

    # shift s -> column group; g0 gets 7 shifts, g1 6 (+bx), g2/g3 6.
    grp_of = [s % 4 for s in range(25)]
    order = []  # round-robin issue order for concurrency
    by_g = [[s for s in range(25) if grp_of[s] == g] for g in range(4)]
    for r in range(7):
        for g in range(4):
            if r < len(by_g[g]):
                order.append(by_g[g][r])

    with TileContext(nc) as tc:
        with tc.tile_pool(name="const", bufs=1) as cpool, \
             tc.tile_pool(name="state", bufs=1) as spool, \
             tc.tile_pool(name="work", bufs=3) as wpool, \
             tc.tile_pool(name="psum", bufs=3, space="PSUM") as ppool:
            wc = cpool.tile([F, 75], f32, tag="wc")
            nc.sync.dma_start(wc[:], wc_d[:])
            wcomb = cpool.tile([100, F], f32, tag="wcomb")
            nc.sync.dma_start(wcomb[:], wcomb_d[:])
            bx = cpool.tile([C, NPOS], f32, tag="bx")
            nc.sync.dma_start(bx[:], bx_d[:])
            i3 = cpool.tile([C, C], f32, tag="i3")
            nc.sync.dma_start(i3[:], i3_d[:])
            lam2 = cpool.tile([F, 2], f32, tag="lam")
            nc.sync.dma_start(lam2[:], lam_d[:])
            bet = cpool.tile([F, nbet], f32, tag="bet")
            nc.sync.dma_start(bet[:], bet_d[:])

            # zy rows 0:75 = y_tmp (padded); rows 96:99 = zcomb; row 99 = ones.
            # One tile so the At-phase is a single K=100 matmul.
            zy = spool.tile([100, NP2 * NP2], f32, tag="zy")
            yn = [spool.tile([F, NPOS], f32, tag=f"yn{i}", name=f"yn{i}")
                  for i in range(2)]
            nc.vector.memset(zy[:], 0.0)
            nc.vector.memset(zy[96:100, :], 1.0)  # row 99 stays 1.0
            nc.vector.memset(yn[0][:], 0.0)
            nc.vector.memset(yn[1][:], 0.0)

            zyv = zy[:].rearrange("p (r c) -> p r c", c=NP2)
            ytv = zyv[0:F]
            yt_int = ytv[:, 2:2 + NB, 2:2 + NB]
            zc_int = zyv[96:99, 2:2 + NB, 2:2 + NB]
            comb_int = zyv[:, 2:2 + NB, 2:2 + NB]

            def iteration(parity, b_ap, b1_ap):
                """One FISTA iteration. parity: ynew index. b_ap / b1_ap:
                beta and 1+beta as [75,1] APs (or float immediates)."""
                ynew = yn[parity]
                ylast = yn[1 - parity]

                # early: e = beta * y_last (overlaps z-phase)
                e = wpool.tile([F, NPOS], f32, tag="e")
                nc.vector.tensor_scalar_mul(e[:], ylast[:], b_ap)

                # z-phase: a~ = A_lin(y_tmp) + bx in 4 PSUM column groups
                pz = ppool.tile([128, NPOS], f32, tag="pz")
                nc.tensor.matmul(pz[32:35, :], i3[:], bx[:], start=True,
                                 stop=False, tile_position=(0, 32))
                seen = [0, 0, 0, 0]
                for s in order:
                    g = grp_of[s]
                    m, nn_ = divmod(s, 5)
                    nc.tensor.matmul(
                        pz[32 * g:32 * g + 3, :],
                        wc[:, 3 * s:3 * s + 3],
                        ytv[:, m:m + NB, nn_:nn_ + NB],
                        start=(seen[g] == 0 and g != 1),
                        stop=(seen[g] == len(by_g[g]) - 1),
                        tile_position=(0, 32 * g),
                    )
                    seen[g] += 1

                # reduce groups: zc = (P0+P1) + (P2+P3)
                h1 = wpool.tile([3, NPOS], f32, tag="h1")
                nc.scalar.copy(h1[:], pz[32:35, :])
                h2 = wpool.tile([3, NPOS], f32, tag="h2")
                nc.vector.tensor_copy(h2[:], pz[96:99, :])
                s1 = wpool.tile([3, NPOS], f32, tag="s1")
                nc.vector.tensor_add(s1[:], pz[0:3, :], h1[:])
                s2 = wpool.tile([3, NPOS], f32, tag="s2")
                nc.vector.tensor_add(s2[:], pz[64:67, :], h2[:])
                nc.vector.tensor_add(zc_int, s1[:], s2[:])

                # At-phase: w = Wcomb^T zy_int as ONE K=100 matmul
                pw = ppool.tile([F, NPOS], f32, tag="pw")
                nc.tensor.matmul(pw[:], wcomb[:], comb_int, start=True, stop=True)

                # soft threshold (reads PSUM)
                cl = wpool.tile([F, NPOS], f32, tag="cl")
                nc.vector.tensor_scalar(
                    cl[:], pw[:], lam2[:, 0:1], lam2[:, 1:2], Alu.min, Alu.max
                )
                nc.vector.tensor_sub(ynew[:], pw[:], cl[:])

                # momentum: y_tmp' = (1+beta)*y_new - e, fused
                nc.vector.scalar_tensor_tensor(
                    yt_int, ynew[:], b1_ap, e[:], Alu.mult, Alu.subtract
                )

            if n_loop > 0:
                # loop var j = 2*n (beta-table column units), steps 2*UNROLL
                with tc.For_i(0, 2 * n_loop, 2 * UNROLL) as j:
                    for u in range(UNROLL):
                        iteration(u % 2, bet[:, ds(j + 2 * u, 1)],
                                  bet[:, ds(j + 2 * u + 1, 1)])
            for n in range(n_loop, n_iters):
                iteration(n % 2, betas[n], 1.0 + betas[n])

            nc.sync.dma_start(y_d[:], yn[(n_iters - 1) % 2][:])

    nc.compile()
    _DEV_CACHE[key] = nc
    return nc


def _bet_table(n_iters, nbet_min=2):
    nbet = max(2 * n_iters, 2, nbet_min)
    bet = np.zeros((F, nbet), np.float32)
    if n_iters > 0:
        b = np.asarray(_betas(n_iters), np.float32)
        bet[:, 0:2 * n_iters:2] = b[None, :]
        bet[:, 1:2 * n_iters:2] = (np.float32(1.0) + b)[None, :]
    return bet


def kernel(x, lam, w_conv, b_conv, w_ct, b_ct):
    from concourse import bass_utils

    x = np.asarray(x, np.float32)
    lam = np.asarray(lam, np.float32)
    w_conv = np.asarray(w_conv, np.float32)
    b_conv = np.asarray(b_conv, np.float32)
    w_ct = np.asarray(w_ct, np.float32)
    b_ct = np.asarray(b_ct, np.float32)
    B = x.shape[0]

    # ---- host analysis (exact): c = At(x - b_conv) + b_ct on the live grid
    w_rev = w_ct[::-1, ::-1]                      # [a,b,ci,co] = w_ct[4-a,4-b,ci,co]
    xb = x - b_conv                               # (B,9,9,3)
    # c[s, a, b, co, I, J]
    c = np.einsum('abeo,sije->sabo' 'ij', w_rev, xb, optimize=True)
    c = c + b_ct[None, None, None, :, None, None]
    cmax = np.abs(c).max(axis=(1, 2, 3, 4, 5))
    active = cmax > lam * np.float32(1.0 - 1e-5)
    act_idx = np.where(active)[0]

    # ---- device weights (same for every core)
    aa, bb_, cc = np.meshgrid(np.arange(5), np.arange(5), np.arange(C), indexing='ij')
    # Wc_all[f=(a,b,ci), 3*s+co] = w_conv[8m+a, 8n+b, ci, co],  s = 5m+n
    Wc_all = np.zeros((F, 75), np.float32)
    for s in range(25):
        m, n = divmod(s, 5)
        blk = w_conv[8 * m + aa, 8 * n + bb_, cc, :]      # (5,5,3,3)
        Wc_all[:, 3 * s:3 * s + 3] = blk.reshape(F, C)
    # Wcomb: rows 0:75 identity (adds y_tmp); rows 96:99 = Wr (At weights,
    # [ci, (a,b,co)] = w_rev[a,b,ci,co]); row 99 = -b_ct (ones row in zy).
    Wcomb = np.zeros((100, F), np.float32)
    Wcomb[0:F, :] = np.eye(F, dtype=np.float32)
    Wcomb[96:99, :] = np.transpose(w_rev, (2, 0, 1, 3)).reshape(C, F)
    Wcomb[99, :] = np.broadcast_to(-b_ct, (5, 5, C)).reshape(F)
    I3 = np.eye(C, dtype=np.float32)
    bet = _bet_table(N_ITERS)

    out = np.zeros((B, HW, HW, C), np.float32)

    # Non-patch positions evolve autonomously: w = y - b_ct per channel.
    # Exact when b_ct == 0 (it is, per the model); otherwise computed here.
    if np.any(b_ct != 0.0):
        betas = _betas(N_ITERS)
        yv = np.zeros((B, C), np.float32)
        yl = np.zeros((B, C), np.float32)
        for n in range(N_ITERS):
            w_np = yv - b_ct[None, :]
            y_new = (np.maximum(w_np - lam[:, None], 0)
                     - np.maximum(-w_np - lam[:, None], 0)).astype(np.float32)
            yv = y_new + np.float32(betas[n]) * (y_new - yl)
            yl = y_new
        mask = np.ones((HW, HW), bool)
        rows = (np.arange(HW) % 8) < 5
        mask[np.ix_(rows, rows)] = False          # live-grid positions
        out[:, mask, :] = yl[:, None, :]

    nc = _build_device(N_ITERS)

    n_rounds = max(1, math.ceil(len(act_idx) / N_CORES))
    zero_bx = np.zeros((C, NPOS), np.float32)
    one_lam = np.stack([np.ones(F, np.float32), -np.ones(F, np.float32)], axis=1)
    for r in range(n_rounds):
        batch = act_idx[r * N_CORES:(r + 1) * N_CORES]
        in_maps = []
        for k in range(N_CORES):
            if k < len(batch):
                s = int(batch[k])
                bxs = np.ascontiguousarray(
                    (b_conv[:, None] - x[s].reshape(NPOS, C).T).astype(np.float32))
                lam2 = np.stack([np.full(F, lam[s], np.float32),
                                 np.full(F, -lam[s], np.float32)], axis=1)
            else:
                bxs, lam2 = zero_bx, one_lam
            in_maps.append({
                "wc": Wc_all, "wcomb": Wcomb, "bx": bxs, "i3": I3,
                "lam2": np.ascontiguousarray(lam2), "bet": bet,
            })
        res = bass_utils.run_bass_kernel_spmd(nc, in_maps, core_ids=list(range(N_CORES)))
        for k in range(len(batch)):
            s = int(batch[k])
            ya = res.results[k]["y"].reshape(5, 5, C, NB, NB)
            # out[s, 8I+a, 8J+b, ci] = ya[a,b,ci,I,J]
            blk = np.transpose(ya, (3, 0, 4, 1, 2))   # (I,a,J,b,ci)
            ov = out[s].reshape(NB, 8, NB, 8, C)
            ov[:, :5, :, :5, :] = blk
    return out


# revision 21
# speedup vs baseline: 2.2916x; 2.2916x over previous
"""Trainium2 Bass kernel for nn_CompressedSensingConvolutional.

Problem: 200 FISTA iterations of
    re    = conv_transpose(x - conv(y_tmp, w_conv, stride 8, SAME) - b_conv,
                           w_ct, stride 8, SAME) + b_ct
    w     = y_tmp - re
    y_new = soft_threshold(w, lam)        (per-sample lam)
    y_tmp = y_new + beta_n (y_new - y_last)
with x: (64,9,9,3), output y_new: (64,72,72,3).

Structure exploited (all exact, no approximations):
  * conv_transpose with 5x5 kernel / stride 8 writes NON-overlapping 5x5
    patches at output rows/cols 8I+a, a in 0..4. Positions with row%8>4 or
    col%8>4 never receive an update, so (given b_ct=0 there) they stay 0
    bitwise forever. The live state is a 45x45x3 = [75=(a,b,ci), 9x9 blocks]
    phase-space grid of 6075 values per sample.
  * With y==0, w = -c where c = At(x - b_conv) + b_ct. If |c| <= lam
    elementwise, soft_threshold returns exactly 0 and the state never
    leaves 0: the sample's output is exactly zero. Only samples with
    max|c| > lam ("active") need the 200-iteration loop at all.
  * conv(y) restricted to the live grid is a 5x5 conv over the 9x9 block
    grid with 75 input channels -> 3 outputs = 25 shift-matmuls (K=75, M=3,
    N=81) accumulated in PSUM.
  * FISTA momentum commutes with the linear conv: A(y_new + b(y_new-y_last))
    is formed in the tiny 3x81 z-space from per-iteration conv outputs,
    so y_tmp is never materialized for the conv input.

Each active sample runs on its own NeuronCore (8 cores; extra actives are
handled in additional device rounds). The 200 iterations run inside a
tc.For_i HARDWARE LOOP (4 iterations per body; y ping-pong stays static)
with per-iteration momentum coefficients read from an SBUF table via
dynamic slices and staggered semaphore resets at the back edge (no
all-engine drain barriers, so adjacent iterations overlap). The static
program is ~180 instructions instead of ~7400 fully unrolled, which
removes the NEFF-size-proportional per-launch cost that dominated the
unrolled version; measured device time is ~2.6 us/iteration (the four
tile_position column streams genuinely overlap on the PE array -- a
1-group variant measures 3.3x slower).
"""

import math
import os
import sys

import numpy as np

for _p in ("/opt/trn_rl_repo", "/root/.axon_site/_ro/trn_rl_repo"):
    if os.path.isdir(_p) and _p not in sys.path:
        sys.path.insert(0, _p)

N_ITERS = 200
N_CORES = 8
HW = 72
LOW = 9
C = 3
F = 75          # (a,b,ci): 5*5*3 live phase-space channels
NP2 = 13        # padded block grid (9 + 2 on each side)
NB = 9          # block grid
NPOS = NB * NB  # 81
KZP = 99        # partial-collection tile: quadrant starts 0/32/64/96 + 3
UNROLL = 4      # iterations per hw-loop body (even: keeps ping-pong static)


def _betas(n_iters):
    """beta_n = (t_n - 1)/t_{n+1}, bit-exact fp32 mirror of the reference."""
    one, two, four = np.float32(1.0), np.float32(2.0), np.float32(4.0)
    t = np.float32(1.0)
    out = []
    for _ in range(n_iters):
        t_n = (one + np.sqrt(one + four * t * t)) / two
        out.append(float((t - one) / t_n))
        t = t_n
    return out


_DEV_CACHE = {}


def _build_device(n_iters, nbet_min=2, no_loop=False, groups=4,
                  staggered=True):
    """Per-core FISTA program (SPMD, same code all cores).

    Layout: one active sample per core. State y_tmp lives in a padded
    [75, 13x13] fp32 SBUF tile. Per iteration:
      z-phase : 25 shift-matmuls (5x5 phase conv, K=75, M=3, N=81) spread
                over 4 PSUM column groups (tile_position col packing) so 4
                streams overlap on the PE array; +1 matmul adds bx.
      reduce  : group partials summed into zy's zcomb rows - 2 copies
                (1 on ACT) + 3 adds on DVE.
      At-phase: w = Wcomb^T zy_int as ONE K=100 matmul (identity rows fold
                in y_tmp, Wr rows apply At, ones row folds -b_ct).
      soft    : cl = clamp(w, +-lam); y_new = w - cl   (DVE, reads PSUM)
      momentum: y_tmp' = (1+beta_n) y_new - beta_n y_last, with
                e = beta_n*y_last issued early (overlaps the z-phase) and
                the rest fused into one scalar_tensor_tensor op.

    ``nbet_min`` pads the beta-table input so different n_iters builds can
    share identical input shapes (used by the timing harness).
    ``no_loop`` forces a fully python-unrolled build (used for the
    TimelineSim cost model, which cannot simulate register branches).
    """
    key = (n_iters, nbet_min, no_loop, groups, staggered)
    if key in _DEV_CACHE:
        return _DEV_CACHE[key]

    import concourse.bacc as bacc
    import concourse.mybir as mybir
    from concourse.bass import ds
    from concourse.tile import TileContext

    f32 = mybir.dt.float32
    Alu = mybir.AluOpType

    betas = [float(b) for b in _betas(max(n_iters, 1))]
    n_loop = 0 if no_loop else (n_iters // UNROLL) * UNROLL
    nbet = max(2 * n_iters, 2, nbet_min)

    nc = bacc.Bacc(trn_type="TRN2")
    wc_d = nc.dram_tensor("wc", [F, 75], f32, kind="ExternalInput")
    wid_d = nc.dram_tensor("wid", [F, F], f32, kind="ExternalInput")
    kzp = KZP if groups == 4 else 4
    wzp_d = nc.dram_tensor("wzp", [kzp, F], f32, kind="ExternalInput")
    bx_d = nc.dram_tensor("bx", [C, NPOS], f32, kind="ExternalInput")
    i3_d = nc.dram_tensor("i3", [C, C], f32, kind="ExternalInput")
    lam_d = nc.dram_tensor("lam2", [F, 2], f32, kind="ExternalInput")
    bet_d = nc.dram_tensor("bet", [F, nbet], f32, kind="ExternalInput")
    y_d = nc.dram_tensor("y", [F, NPOS], f32, kind="ExternalOutput")

    # shift s -> column group; g0 gets 7 shifts, g1 6 (+bx), g2/g3 6.
    grp_of = [s % groups for s in range(25)]
    order = []  # round-robin issue order for concurrency
    by_g = [[s for s in range(25) if grp_of[s] == g] for g in range(groups)]
    for r in range(25):
        for g in range(groups):
            if r < len(by_g[g]):
                order.append(by_g[g][r])

    with TileContext(nc) as tc:
        with tc.tile_pool(name="const", bufs=1) as cpool, \
             tc.tile_pool(name="state", bufs=1) as spool, \
             tc.tile_pool(name="work", bufs=3) as wpool, \
             tc.tile_pool(name="psum", bufs=3, space="PSUM") as ppool:
            wc = cpool.tile([F, 75], f32, tag="wc")
            nc.sync.dma_start(wc[:], wc_d[:])
            wid = cpool.tile([F, F], f32, tag="wid")
            nc.sync.dma_start(wid[:], wid_d[:])
            wzp = cpool.tile([kzp, F], f32, tag="wzp")
            nc.sync.dma_start(wzp[:], wzp_d[:])
            bx = cpool.tile([C, NPOS], f32, tag="bx")
            nc.sync.dma_start(bx[:], bx_d[:])
            i3 = cpool.tile([C, C], f32, tag="i3")
            nc.sync.dma_start(i3[:], i3_d[:])
            lam2 = cpool.tile([F, 2], f32, tag="lam")
            nc.sync.dma_start(lam2[:], lam_d[:])
            bet = cpool.tile([F, nbet], f32, tag="bet")
            nc.sync.dma_start(bet[:], bet_d[:])

            # zy = padded y_tmp only. zp collects the four z-space group
            # partials at QUADRANT-ALIGNED partition starts (0/32/64/96 --
            # engine APs must start at 32-aligned partitions); row 3 is a
            # constant-ones row (garbage rows 4:32 carry zero At weights).
            zy = spool.tile([F, NP2 * NP2], f32, tag="zy")
            zp = spool.tile([kzp, NPOS], f32, tag="zp")
            yn = [spool.tile([F, NPOS], f32, tag=f"yn{i}", name=f"yn{i}")
                  for i in range(2)]
            nc.vector.memset(zy[:], 0.0)
            nc.vector.memset(zp[:], 0.0)
            nc.vector.memset(zp[0:min(32, kzp), :], 1.0)  # row 3 = ones row
            nc.vector.memset(yn[0][:], 0.0)
            nc.vector.memset(yn[1][:], 0.0)

            zyv = zy[:].rearrange("p (r c) -> p r c", c=NP2)
            ytv = zyv[0:F]
            yt_int = ytv[:, 2:2 + NB, 2:2 + NB]

            def iteration(parity, b_ap, b1_ap):
                """One FISTA iteration. parity: ynew index. b_ap / b1_ap:
                beta and 1+beta as [75,1] APs (or float immediates)."""
                ynew = yn[parity]
                ylast = yn[1 - parity]

                # early: e = beta * y_last (overlaps z-phase)
                e = wpool.tile([F, NPOS], f32, tag="e")
                nc.vector.tensor_scalar_mul(e[:], ylast[:], b_ap)

                # z-phase: a~ = A_lin(y_tmp) + bx in 4 PSUM column groups
                pz = ppool.tile([128, NPOS], f32, tag="pz")
                bxg = 1 if groups == 4 else 0
                nc.tensor.matmul(pz[32 * bxg:32 * bxg + 3, :], i3[:], bx[:],
                                 start=True, stop=False,
                                 tile_position=(0, 32 * bxg))
                seen = [0] * groups
                for s in order:
                    g = grp_of[s]
                    m, nn_ = divmod(s, 5)
                    nc.tensor.matmul(
                        pz[32 * g:32 * g + 3, :],
                        wc[:, 3 * s:3 * s + 3],
                        ytv[:, m:m + NB, nn_:nn_ + NB],
                        start=(seen[g] == 0 and g != bxg),
                        stop=(seen[g] == len(by_g[g]) - 1),
                        tile_position=(0, 32 * g),
                    )
                    seen[g] += 1

                # move partials to zp (quadrant-aligned); the At-phase
                # matmul reduces them inside PSUM accumulation -- no DVE
                # adds. Copies spread over DVE/ACT/GPSIMD run concurrently,
                # overlapped by the y-identity matmul below.
                if groups == 4:
                    nc.vector.tensor_copy(zp[0:3, :], pz[0:3, :])
                    nc.scalar.copy(zp[32:35, :], pz[32:35, :])
                    nc.scalar.copy(zp[64:67, :], pz[64:67, :])
                    nc.vector.tensor_copy(zp[96:99, :], pz[96:99, :])
                else:
                    nc.vector.tensor_copy(zp[0:3, :], pz[0:3, :])

                # At-phase: w = y_tmp + Wzp^T zp, two accumulating matmuls.
                # The K=75 identity matmul needs only y_tmp, so it runs
                # while the copies are still in flight.
                pw = ppool.tile([F, NPOS], f32, tag="pw")
                nc.tensor.matmul(pw[:], wid[:], yt_int, start=True, stop=False)
                nc.tensor.matmul(pw[:], wzp[:], zp[:], start=False, stop=True)

                # soft threshold (reads PSUM)
                cl = wpool.tile([F, NPOS], f32, tag="cl")
                nc.vector.tensor_scalar(
                    cl[:], pw[:], lam2[:, 0:1], lam2[:, 1:2], Alu.min, Alu.max
                )
                nc.vector.tensor_sub(ynew[:], pw[:], cl[:])

                # momentum: y_tmp' = (1+beta)*y_new - e, fused
                nc.vector.scalar_tensor_tensor(
                    yt_int, ynew[:], b1_ap, e[:], Alu.mult, Alu.subtract
                )

            if n_loop > 0:
                # loop var j = 2*n (beta-table column units), steps 2*UNROLL
                with tc.For_i(0, 2 * n_loop, 2 * UNROLL,
                              staggered_reset=staggered) as j:
                    for u in range(UNROLL):
                        iteration(u % 2, bet[:, ds(j + 2 * u, 1)],
                                  bet[:, ds(j + 2 * u + 1, 1)])
            for n in range(n_loop, n_iters):
                iteration(n % 2, betas[n], 1.0 + betas[n])

            nc.sync.dma_start(y_d[:], yn[(n_iters - 1) % 2][:])

    nc.compile()
    _DEV_CACHE[key] = nc
    return nc


def _bet_table(n_iters, nbet_min=2):
    nbet = max(2 * n_iters, 2, nbet_min)
    bet = np.zeros((F, nbet), np.float32)
    if n_iters > 0:
        b = np.asarray(_betas(n_iters), np.float32)
        bet[:, 0:2 * n_iters:2] = b[None, :]
        bet[:, 1:2 * n_iters:2] = (np.float32(1.0) + b)[None, :]
    return bet


def kernel(x, lam, w_conv, b_conv, w_ct, b_ct):
    from concourse import bass_utils

    x = np.asarray(x, np.float32)
    lam = np.asarray(lam, np.float32)
    w_conv = np.asarray(w_conv, np.float32)
    b_conv = np.asarray(b_conv, np.float32)
    w_ct = np.asarray(w_ct, np.float32)
    b_ct = np.asarray(b_ct, np.float32)
    B = x.shape[0]

    # ---- host analysis (exact): c = At(x - b_conv) + b_ct on the live grid
    w_rev = w_ct[::-1, ::-1]                      # [a,b,ci,co] = w_ct[4-a,4-b,ci,co]
    xb = x - b_conv                               # (B,9,9,3)
    # c[s, a, b, co, I, J]
    c = np.einsum('abeo,sije->sabo' 'ij', w_rev, xb, optimize=True)
    c = c + b_ct[None, None, None, :, None, None]
    cmax = np.abs(c).max(axis=(1, 2, 3, 4, 5))
    active = cmax > lam * np.float32(1.0 - 1e-5)
    act_idx = np.where(active)[0]

    # ---- device weights (same for every core)
    aa, bb_, cc = np.meshgrid(np.arange(5), np.arange(5), np.arange(C), indexing='ij')
    # Wc_all[f=(a,b,ci), 3*s+co] = w_conv[8m+a, 8n+b, ci, co],  s = 5m+n
    Wc_all = np.zeros((F, 75), np.float32)
    for s in range(25):
        m, n = divmod(s, 5)
        blk = w_conv[8 * m + aa, 8 * n + bb_, cc, :]      # (5,5,3,3)
        Wc_all[:, 3 * s:3 * s + 3] = blk.reshape(F, C)
    # Wzp: Wr = At weights ([ci, (a,b,co)] = w_rev[a,b,ci,co]) at each
    # quadrant-aligned partial block; row 3 = -b_ct (ones row in zp).
    Wr = np.transpose(w_rev, (2, 0, 1, 3)).reshape(C, F)
    Wzp = np.zeros((KZP, F), np.float32)
    for q in range(4):
        Wzp[32 * q:32 * q + 3, :] = Wr
    Wzp[3, :] = np.broadcast_to(-b_ct, (5, 5, C)).reshape(F)
    Wid = np.eye(F, dtype=np.float32)
    I3 = np.eye(C, dtype=np.float32)
    bet = _bet_table(N_ITERS)

    out = np.zeros((B, HW, HW, C), np.float32)

    # Non-patch positions evolve autonomously: w = y - b_ct per channel.
    # Exact when b_ct == 0 (it is, per the model); otherwise computed here.
    if np.any(b_ct != 0.0):
        betas = _betas(N_ITERS)
        yv = np.zeros((B, C), np.float32)
        yl = np.zeros((B, C), np.float32)
        for n in range(N_ITERS):
            w_np = yv - b_ct[None, :]
            y_new = (np.maximum(w_np - lam[:, None], 0)
                     - np.maximum(-w_np - lam[:, None], 0)).astype(np.float32)
            yv = y_new + np.float32(betas[n]) * (y_new - yl)
            yl = y_new
        mask = np.ones((HW, HW), bool)
        rows = (np.arange(HW) % 8) < 5
        mask[np.ix_(rows, rows)] = False          # live-grid positions
        out[:, mask, :] = yl[:, None, :]

    nc = _build_device(N_ITERS)

    n_rounds = max(1, math.ceil(len(act_idx) / N_CORES))
    zero_bx = np.zeros((C, NPOS), np.float32)
    one_lam = np.stack([np.ones(F, np.float32), -np.ones(F, np.float32)], axis=1)
    for r in range(n_rounds):
        batch = act_idx[r * N_CORES:(r + 1) * N_CORES]
        in_maps = []
        for k in range(N_CORES):
            if k < len(batch):
                s = int(batch[k])
                bxs = np.ascontiguousarray(
                    (b_conv[:, None] - x[s].reshape(NPOS, C).T).astype(np.float32))
                lam2 = np.stack([np.full(F, lam[s], np.float32),
                                 np.full(F, -lam[s], np.float32)], axis=1)
            else:
                bxs, lam2 = zero_bx, one_lam
            in_maps.append({
                "wc": Wc_all, "wid": Wid, "wzp": Wzp, "bx": bxs, "i3": I3,
                "lam2": np.ascontiguousarray(lam2), "bet": bet,
            })
        res = bass_utils.run_bass_kernel_spmd(nc, in_maps, core_ids=list(range(N_CORES)))
        for k in range(len(batch)):
            s = int(batch[k])
            ya = res.results[k]["y"].reshape(5, 5, C, NB, NB)
            # out[s, 8I+a, 8J+b, ci] = ya[a,b,ci,I,J]
            blk = np.transpose(ya, (3, 0, 4, 1, 2))   # (I,a,J,b,ci)
            ov = out[s].reshape(NB, 8, NB, 8, C)
            ov[:, :5, :, :5, :] = blk
    return out
